# revision 9
# baseline (speedup 1.0000x reference)
"""DynamicKLDiscretLoss on 8 Trainium2 NeuronCores (Bass/Tile).

Data-parallel: batch dim (2048) sharded 8 ways -> 256 batches/core.
Each core computes its partial weighted loss sum; host adds the 8 partials.

Per [128-row, W] tile, per tensor (output_x/target_x W=384, output_y/target_y W=512):
  - exact sorted top-k (k=W/4) via iterative DVE max8 + match_replace rounds
  - tiny MLP (topk++mean -> relu -> sigmoid) on PE/ACT to get per-row beta
  - softmax-free KL:  loss_row = (bg*A - bp*B)/Zg + (lnZp - lnZg)/W
      A = (1/W) sum e*gt, B = (1/W) sum e*pred, e = exp(gt*bg), Z* = sum exp(l*)
    (exact algebraic rewrite of mean(labels*(log_labels-scores)); no max-sub
     needed since |logits| <= ~11 in fp32)
"""

import sys

sys.path.insert(0, "/opt/trn_rl_repo")

from contextlib import ExitStack

import numpy as np

import concourse.bass as bass
import concourse.tile as tile
from concourse import mybir
from concourse.bass_utils import run_bass_kernel_spmd
from concourse.masks import make_identity

F32 = mybir.dt.float32
AF = mybir.ActivationFunctionType

B, K, WX, WY = 2048, 17, 384, 512
NCORES = 8
BP = B // NCORES          # 256 batches per core
ROWS = BP * K             # 4352 rows per core
P = 128
NT = ROWS // P            # 34 tiles per core
NEG = -1.0e30

# tensor order everywhere: 0=output_x(pred,x) 1=output_y(pred,y) 2=target_x(gt,x) 3=target_y(gt,y)
TENSORS = [
    ("output_x", WX), ("output_y", WY), ("target_x", WX), ("target_y", WY),
]

# walrus in this container rejects >1 sync wait per instruction; Tile's
# semaphore pass emits multi-wait instructions (the tail drain always does).
MAX_WAITS = 1


def split_excess_waits(nc):
    ctr = 0
    for func in nc.m.functions:
        for block in func.blocks:
            insts = list(block.instructions)
            out_list, changed = [], False
            for inst in insts:
                si = inst.sync_info
                if si is not None and si.on_wait and len(si.on_wait) > MAX_WAITS:
                    w = list(si.on_wait)
                    si.on_wait = w[:MAX_WAITS]
                    rest = w[MAX_WAITS:]
                    while rest:
                        chunk, rest = rest[:MAX_WAITS], rest[MAX_WAITS:]
                        ctr += 1
                        nop = mybir.InstNoOp(name=f"I-wfix-{ctr}", ins=[], outs=[])
                        nop.engine = inst.engine
                        nop.sync_info = mybir.SyncInfo(on_wait=chunk, on_update=[])
                        out_list.append(nop)
                    changed = True
                out_list.append(inst)
            if changed:
                block.instructions = out_list
    return ctr


def build_nc():
    nc = bass.Bass()

    d = {}
    for name, w in TENSORS:
        d[name] = nc.dram_tensor(name, [ROWS, w], F32, kind="ExternalInput")
    d["tw"] = nc.dram_tensor("tw", [P, NT], F32, kind="ExternalInput")
    for pre, w in (("fcx", WX), ("fcy", WY)):
        kk, hh = w // 4, w // 8
        d[f"{pre}_w1"] = nc.dram_tensor(f"{pre}_w1", [kk + 1, hh], F32, kind="ExternalInput")
        d[f"{pre}_b1"] = nc.dram_tensor(f"{pre}_b1", [hh, 1], F32, kind="ExternalInput")
        d[f"{pre}_w2"] = nc.dram_tensor(f"{pre}_w2", [hh, 1], F32, kind="ExternalInput")
        d[f"{pre}_b2"] = nc.dram_tensor(f"{pre}_b2", [1, 1], F32, kind="ExternalInput")
    out_d = nc.dram_tensor("out", [1, 1], F32, kind="ExternalOutput")

    with tile.TileContext(nc) as tc, ExitStack() as ctx:
        singles = ctx.enter_context(tc.tile_pool(name="singles", bufs=1))
        io = ctx.enter_context(tc.tile_pool(name="io", bufs=3))
        work = ctx.enter_context(tc.tile_pool(name="work", bufs=2))
        psT = ctx.enter_context(tc.tile_pool(name="psT", bufs=2, space="PSUM"))
        psS = ctx.enter_context(tc.tile_pool(name="psS", bufs=1, space="PSUM"))

        ident = singles.tile([P, P], F32)
        make_identity(nc, ident)
        ones = singles.tile([P, 1], F32)
        nc.vector.memset(ones, 1.0)
        winv2 = singles.tile([P, 2], F32)
        nc.vector.memset(winv2[:, 0:1], 1.0 / WX)
        nc.vector.memset(winv2[:, 1:2], 1.0 / WY)
        tw = singles.tile([P, NT], F32)
        nc.sync.dma_start(out=tw, in_=d["tw"][:, :])
        acc = singles.tile([P, NT], F32)

        # weights: per branch (x, y)
        wts = {}
        for bi, (pre, w) in enumerate((("fcx", WX), ("fcy", WY))):
            kk, hh = w // 4, w // 8
            w1m = singles.tile([kk, hh], F32, tag=f"w1m{bi}")
            nc.sync.dma_start(out=w1m, in_=d[f"{pre}_w1"][0:kk, :])
            w1u = singles.tile([1, hh], F32, tag=f"w1u{bi}")
            nc.sync.dma_start(out=w1u, in_=d[f"{pre}_w1"][kk : kk + 1, :])
            b1 = singles.tile([hh, 1], F32, tag=f"b1{bi}")
            nc.sync.dma_start(out=b1, in_=d[f"{pre}_b1"][:, :])
            w2 = singles.tile([hh, 1], F32, tag=f"w2{bi}")
            nc.sync.dma_start(out=w2, in_=d[f"{pre}_w2"][:, :])
            b2 = singles.tile([1, 1], F32, tag=f"b2{bi}")
            nc.sync.dma_start(out=b2, in_=d[f"{pre}_b2"][:, :])
            wts[bi] = (w1m, w1u, b1, w2, b2)

        for t in range(NT):
            xt, cp = {}, {}
            means = work.tile([P, 4], F32, tag="means")
            tkT_sb = {}
            for i, (name, w) in enumerate(TENSORS):
                kk = w // 4
                xt[i] = io.tile([P, w], F32, tag=f"in{i}", name=f"x{i}")
                nc.sync.dma_start(out=xt[i], in_=d[name][t * P : (t + 1) * P, :])
                # copy (for destructive topk) + row-sum in one ACT pass
                cp[i] = work.tile([P, w], F32, tag=f"cp{i}", name=f"c{i}")
                nc.scalar.activation(
                    out=cp[i], in_=xt[i], func=AF.Copy,
                    accum_out=means[:, i : i + 1],
                )
                tk = work.tile([P, kk], F32, tag=f"tk{i}")
                nr = kk // 8
                for r in range(nr):
                    nc.vector.max(tk[:, 8 * r : 8 * r + 8], cp[i][:, :])
                    if r + 1 < nr:
                        nc.vector.match_replace(
                            cp[i][:, :], tk[:, 8 * r : 8 * r + 8], cp[i][:, :], NEG
                        )
                tkT_ps = psT.tile([kk, P], F32, tag="tkT")
                nc.tensor.transpose(tkT_ps, tk, ident)
                tkT_sb[i] = work.tile([kk, P], F32, tag="tkTs", name=f"tkTs{i}")
                nc.scalar.activation(out=tkT_sb[i], in_=tkT_ps, func=AF.Copy)

                # MLP part 1: z = w1'.T @ topk' (+ mean term added after means ready)
                # (done below once means are transposed)

            # matmul rhs / ACT-PSUM reads must start at partition 0 -> one
            # [1,128] transpose per tensor's mean column
            mT = {}
            for i, (_, w) in enumerate(TENSORS):
                mT_ps = psS.tile([1, P], F32, tag="mT", name=f"mTp{i}")
                nc.tensor.transpose(mT_ps, means[:, i : i + 1], ident)
                mT[i] = work.tile([1, P], F32, tag=f"mTs{i}", name=f"mTs{i}")
                nc.scalar.activation(
                    out=mT[i], in_=mT_ps, func=AF.Copy, scale=1.0 / w
                )

            bcol = work.tile([P, 4], F32, tag="bcol")
            for i, (name, w) in enumerate(TENSORS):
                kk, hh = w // 4, w // 8
                bi = 0 if w == WX else 1
                w1m, w1u, b1, w2, b2 = wts[bi]
                z_ps = psS.tile([hh, P], F32, tag="z")
                nc.tensor.matmul(z_ps, lhsT=w1m, rhs=tkT_sb[i], start=True, stop=False)
                nc.tensor.matmul(z_ps, lhsT=w1u, rhs=mT[i], start=False, stop=True)
                hT = work.tile([hh, P], F32, tag="hT")
                nc.scalar.activation(out=hT, in_=z_ps, func=AF.Relu, bias=b1[:, :])
                g_ps = psS.tile([1, P], F32, tag="g")
                nc.tensor.matmul(g_ps, lhsT=w2, rhs=hT, start=True, stop=True)
                b_i = work.tile([1, P], F32, tag="bi", name=f"bi{i}")
                nc.scalar.activation(out=b_i, in_=g_ps, func=AF.Sigmoid, bias=b2[:, :])
                bc_ps = psS.tile([P, 1], F32, tag="bc", name=f"bc{i}")
                nc.tensor.transpose(bc_ps, b_i, ident[:1, :1])
                # beta = 1 + sigmoid(...): fold the +1 into the PSUM->SBUF copy
                nc.scalar.activation(
                    out=bcol[:, i : i + 1], in_=bc_ps, func=AF.Copy, bias=1.0
                )

            # KL phase. Z cols: Zg_x, Zg_y, Zp_x, Zp_y
            Z = work.tile([P, 4], F32, tag="Z")
            A2 = work.tile([P, 2], F32, tag="A2")
            B2 = work.tile([P, 2], F32, tag="B2")
            for b, (ip, ig, w) in enumerate(((0, 2, WX), (1, 3, WY))):
                e = work.tile([P, w], F32, tag=f"e{b}")
                nc.scalar.activation(
                    out=e, in_=xt[ig], func=AF.Exp,
                    scale=bcol[:, ig : ig + 1], accum_out=Z[:, b : b + 1],
                )
                nc.scalar.activation(
                    out=cp[ip], in_=xt[ip], func=AF.Exp,
                    scale=bcol[:, ip : ip + 1], accum_out=Z[:, 2 + b : 3 + b],
                )
                # TensorTensorReduce lowers via InstISA, which this walrus
                # rejects ("ISA wrong length") -> DVE mult + ACT copy-accum
                prodA = work.tile([P, w], F32, tag=f"prod{b}", name=f"prA{b}")
                nc.vector.tensor_mul(prodA, e, xt[ig])
                nc.scalar.activation(
                    out=cp[ig], in_=prodA, func=AF.Copy, scale=1.0 / w,
                    accum_out=A2[:, b : b + 1],
                )
                prodB = work.tile([P, w], F32, tag=f"prod{b}", name=f"prB{b}")
                nc.vector.tensor_mul(prodB, e, xt[ip])
                nc.scalar.activation(
                    out=cp[ip], in_=prodB, func=AF.Copy, scale=1.0 / w,
                    accum_out=B2[:, b : b + 1],
                )

            lnZ = work.tile([P, 4], F32, tag="lnZ")
            nc.scalar.activation(out=lnZ, in_=Z, func=AF.Ln)
            rg = work.tile([P, 2], F32, tag="rg")
            nc.vector.reciprocal(out=rg, in_=Z[:, 0:2])
            ta = work.tile([P, 2], F32, tag="ta")
            nc.vector.tensor_mul(ta, bcol[:, 2:4], A2)      # bg * A
            tb = work.tile([P, 2], F32, tag="tb")
            nc.vector.tensor_mul(tb, bcol[:, 0:2], B2)      # bp * B
            nc.vector.tensor_sub(ta, ta, tb)
            nc.vector.tensor_mul(ta, ta, rg)                # (bgA-bpB)/Zg
            nc.vector.tensor_sub(tb, lnZ[:, 2:4], lnZ[:, 0:2])  # lnZp - lnZg
            nc.vector.tensor_mul(tb, tb, winv2)
            nc.vector.tensor_add(ta, ta, tb)                # loss rows, x|y cols
            lsum = work.tile([P, 1], F32, tag="lsum")
            nc.vector.tensor_add(lsum, ta[:, 0:1], ta[:, 1:2])
            nc.vector.tensor_mul(acc[:, t : t + 1], lsum, tw[:, t : t + 1])

        accv = singles.tile([P, 1], F32)
        nc.vector.reduce_sum(out=accv, in_=acc, axis=mybir.AxisListType.X)
        tot_ps = psS.tile([1, 1], F32, tag="tot")
        nc.tensor.matmul(tot_ps, lhsT=accv, rhs=ones, start=True, stop=True)
        res = singles.tile([1, 1], F32)
        nc.scalar.activation(out=res, in_=tot_ps, func=AF.Copy, scale=1.0 / K)
        nc.sync.dma_start(out=out_d[:, :], in_=res)

    split_excess_waits(nc)
    return nc


_NC_CACHE = {}


def _get_nc():
    if "nc" not in _NC_CACHE:
        _NC_CACHE["nc"] = build_nc()
    return _NC_CACHE["nc"]


def make_in_maps(inputs):
    in_maps = []
    for c in range(NCORES):
        sl = slice(c * BP, (c + 1) * BP)
        m = {
            "output_x": np.ascontiguousarray(
                inputs["output_x"][sl].reshape(ROWS, WX), np.float32),
            "output_y": np.ascontiguousarray(
                inputs["output_y"][sl].reshape(ROWS, WY), np.float32),
            "target_x": np.ascontiguousarray(
                inputs["target_x"][sl].reshape(ROWS, WX), np.float32),
            "target_y": np.ascontiguousarray(
                inputs["target_y"][sl].reshape(ROWS, WY), np.float32),
            "tw": np.ascontiguousarray(
                inputs["target_weight"][sl].reshape(NT, P).T, np.float32),
            "fcx_w1": np.ascontiguousarray(inputs["fcx_w1"], np.float32),
            "fcx_b1": np.ascontiguousarray(inputs["fcx_b1"].reshape(-1, 1), np.float32),
            "fcx_w2": np.ascontiguousarray(inputs["fcx_w2"], np.float32),
            "fcx_b2": np.ascontiguousarray(inputs["fcx_b2"].reshape(1, 1), np.float32),
            "fcy_w1": np.ascontiguousarray(inputs["fcy_w1"], np.float32),
            "fcy_b1": np.ascontiguousarray(inputs["fcy_b1"].reshape(-1, 1), np.float32),
            "fcy_w2": np.ascontiguousarray(inputs["fcy_w2"], np.float32),
            "fcy_b2": np.ascontiguousarray(inputs["fcy_b2"].reshape(1, 1), np.float32),
        }
        in_maps.append(m)
    return in_maps


def kernel(**inputs) -> np.ndarray:
    nc = _get_nc()
    in_maps = make_in_maps(inputs)
    res = run_bass_kernel_spmd(nc, in_maps, core_ids=list(range(NCORES)))
    total = np.float64(0.0)
    for c in range(NCORES):
        total += np.float64(res.results[c]["out"][0, 0])
    return np.float32(total)


# revision 11
# speedup vs baseline: 1.0444x; 1.0444x over previous
"""DynamicKLDiscretLoss on 8 Trainium2 NeuronCores (Bass/Tile).

Data-parallel: batch dim (2048) sharded 8 ways -> 256 batches/core.
Each core computes its partial weighted loss sum; host adds the 8 partials.

Per [128-row, W] tile, per tensor (output_x/target_x W=384, output_y/target_y W=512):
  - exact sorted top-k (k=W/4) via iterative DVE max8 + match_replace rounds
  - tiny MLP (topk++mean -> relu -> 1+sigmoid) on PE/ACT to get per-row beta;
    sigmoid(z) = (1+tanh(z/2))/2 so every per-tile ACT func ({Copy,Relu,Exp,
    Tanh}) lives in ONE act-table set (avoids per-op ACT_TABLE_LOADs)
  - softmax-free KL:  loss_row = (bg*A - bp*B)/Zg + (lnZp - lnZg)/W
      A = (1/W) sum e*gt, B = (1/W) sum e*pred, e = exp(gt*bg), Z* = sum exp(l*)
    (exact algebraic rewrite of mean(labels*(log_labels-scores)); no max-sub
     needed since |logits| <= ~11 in fp32)
Per-row scalars (Z, A, B, beta) are banked into [128, NT, .] buffers and the
loss assembled in one vectorized epilogue.
"""

import sys

sys.path.insert(0, "/opt/trn_rl_repo")

from contextlib import ExitStack

import numpy as np

import concourse.bass as bass
import concourse.tile as tile
from concourse import mybir
from concourse.bass_utils import run_bass_kernel_spmd
from concourse.masks import make_identity

F32 = mybir.dt.float32
AF = mybir.ActivationFunctionType
OP = mybir.AluOpType

B, K, WX, WY = 2048, 17, 384, 512
NCORES = 8
BP = B // NCORES          # 256 batches per core
ROWS = BP * K             # 4352 rows per core
P = 128
NT = ROWS // P            # 34 tiles per core
NEG = -1.0e30

# tensor order everywhere: 0=output_x(pred,x) 1=output_y(pred,y) 2=target_x(gt,x) 3=target_y(gt,y)
TENSORS = [
    ("output_x", WX), ("output_y", WY), ("target_x", WX), ("target_y", WY),
]

# walrus in this container rejects >1 sync wait per instruction; Tile's
# semaphore pass emits multi-wait instructions (the tail drain always does).
MAX_WAITS = 1


def split_excess_waits(nc):
    ctr = 0
    for func in nc.m.functions:
        for block in func.blocks:
            insts = list(block.instructions)
            out_list, changed = [], False
            for inst in insts:
                si = inst.sync_info
                if si is not None and si.on_wait and len(si.on_wait) > MAX_WAITS:
                    w = list(si.on_wait)
                    si.on_wait = w[:MAX_WAITS]
                    rest = w[MAX_WAITS:]
                    while rest:
                        chunk, rest = rest[:MAX_WAITS], rest[MAX_WAITS:]
                        ctr += 1
                        nop = mybir.InstNoOp(name=f"I-wfix-{ctr}", ins=[], outs=[])
                        nop.engine = inst.engine
                        nop.sync_info = mybir.SyncInfo(on_wait=chunk, on_update=[])
                        out_list.append(nop)
                    changed = True
                out_list.append(inst)
            if changed:
                block.instructions = out_list
    return ctr


def build_nc():
    nc = bass.Bass()

    d = {}
    for name, w in TENSORS:
        d[name] = nc.dram_tensor(name, [ROWS, w], F32, kind="ExternalInput")
    d["tw"] = nc.dram_tensor("tw", [P, NT], F32, kind="ExternalInput")
    for pre, w in (("fcx", WX), ("fcy", WY)):
        kk, hh = w // 4, w // 8
        d[f"{pre}_w1"] = nc.dram_tensor(f"{pre}_w1", [kk + 1, hh], F32, kind="ExternalInput")
        d[f"{pre}_b1"] = nc.dram_tensor(f"{pre}_b1", [hh, 1], F32, kind="ExternalInput")
        d[f"{pre}_w2"] = nc.dram_tensor(f"{pre}_w2", [hh, 1], F32, kind="ExternalInput")
        d[f"{pre}_b2"] = nc.dram_tensor(f"{pre}_b2", [1, 1], F32, kind="ExternalInput")
    out_d = nc.dram_tensor("out", [1, 1], F32, kind="ExternalOutput")

    with tile.TileContext(nc) as tc, ExitStack() as ctx:
        singles = ctx.enter_context(tc.tile_pool(name="singles", bufs=1))
        io = ctx.enter_context(tc.tile_pool(name="io", bufs=3))
        work = ctx.enter_context(tc.tile_pool(name="work", bufs=3))
        psT = ctx.enter_context(tc.tile_pool(name="psT", bufs=2, space="PSUM"))
        psS = ctx.enter_context(tc.tile_pool(name="psS", bufs=1, space="PSUM"))

        ident = singles.tile([P, P], F32)
        make_identity(nc, ident)
        ones = singles.tile([P, 1], F32)
        nc.vector.memset(ones, 1.0)
        tw = singles.tile([P, NT], F32)
        nc.sync.dma_start(out=tw, in_=d["tw"][:, :])

        # per-row scalar banks, filled per tile, consumed by the epilogue
        # col order within each group of 4: (Zg_x, Zg_y, Zp_x, Zp_y)
        Zbuf = singles.tile([P, NT, 4], F32)
        Abuf = singles.tile([P, NT, 2], F32)   # (1/W) sum e*gt   (x, y)
        Bbuf = singles.tile([P, NT, 2], F32)   # (1/W) sum e*pred (x, y)
        bcolbuf = singles.tile([P, NT, 4], F32)  # beta, tensor order

        # weights: per branch (x, y)
        wts = {}
        for bi, (pre, w) in enumerate((("fcx", WX), ("fcy", WY))):
            kk, hh = w // 4, w // 8
            w1m = singles.tile([kk, hh], F32, tag=f"w1m{bi}")
            nc.sync.dma_start(out=w1m, in_=d[f"{pre}_w1"][0:kk, :])
            w1u = singles.tile([1, hh], F32, tag=f"w1u{bi}")
            nc.sync.dma_start(out=w1u, in_=d[f"{pre}_w1"][kk : kk + 1, :])
            b1 = singles.tile([hh, 1], F32, tag=f"b1{bi}")
            nc.sync.dma_start(out=b1, in_=d[f"{pre}_b1"][:, :])
            w2 = singles.tile([hh, 1], F32, tag=f"w2{bi}")
            nc.sync.dma_start(out=w2, in_=d[f"{pre}_w2"][:, :])
            b2 = singles.tile([1, 1], F32, tag=f"b2{bi}")
            nc.sync.dma_start(out=b2, in_=d[f"{pre}_b2"][:, :])
            # tanh path needs b2/2
            b2h = singles.tile([1, 1], F32, tag=f"b2h{bi}")
            nc.gpsimd.tensor_scalar_mul(b2h, b2, 0.5)
            wts[bi] = (w1m, w1u, b1, w2, b2h)

        for t in range(NT):
            xt, cp = {}, {}
            means = work.tile([P, 4], F32, tag="means")
            tkT_sb = {}
            for i, (name, w) in enumerate(TENSORS):
                kk = w // 4
                xt[i] = io.tile([P, w], F32, tag=f"in{i}", name=f"x{i}")
                nc.sync.dma_start(out=xt[i], in_=d[name][t * P : (t + 1) * P, :])
                # copy (for destructive topk) + row-sum in one ACT pass
                cp[i] = work.tile([P, w], F32, tag=f"cp{i}", name=f"c{i}")
                nc.scalar.activation(
                    out=cp[i], in_=xt[i], func=AF.Copy,
                    accum_out=means[:, i : i + 1],
                )
                tk = work.tile([P, kk], F32, tag=f"tk{i}")
                nr = kk // 8
                for r in range(nr):
                    nc.vector.max(tk[:, 8 * r : 8 * r + 8], cp[i][:, :])
                    if r + 1 < nr:
                        nc.vector.match_replace(
                            cp[i][:, :], tk[:, 8 * r : 8 * r + 8], cp[i][:, :], NEG
                        )
                tkT_ps = psT.tile([kk, P], F32, tag="tkT")
                nc.tensor.transpose(tkT_ps, tk, ident)
                tkT_sb[i] = work.tile([kk, P], F32, tag="tkTs", name=f"tkTs{i}")
                nc.scalar.activation(out=tkT_sb[i], in_=tkT_ps, func=AF.Copy)

            # matmul rhs / ACT-PSUM reads must start at partition 0 -> one
            # [1,128] transpose per tensor's mean column
            mT = {}
            for i, (_, w) in enumerate(TENSORS):
                mT_ps = psS.tile([1, P], F32, tag="mT", name=f"mTp{i}")
                nc.tensor.transpose(mT_ps, means[:, i : i + 1], ident)
                mT[i] = work.tile([1, P], F32, tag=f"mTs{i}", name=f"mTs{i}")
                nc.scalar.activation(
                    out=mT[i], in_=mT_ps, func=AF.Copy, scale=1.0 / w
                )

            for i, (name, w) in enumerate(TENSORS):
                kk, hh = w // 4, w // 8
                bi = 0 if w == WX else 1
                w1m, w1u, b1, w2, b2h = wts[bi]
                z_ps = psS.tile([hh, P], F32, tag="z", bufs=2, name=f"z{i}")
                nc.tensor.matmul(z_ps, lhsT=w1m, rhs=tkT_sb[i], start=True, stop=False)
                nc.tensor.matmul(z_ps, lhsT=w1u, rhs=mT[i], start=False, stop=True)
                hT = work.tile([hh, P], F32, tag="hT")
                nc.scalar.activation(out=hT, in_=z_ps, func=AF.Relu, bias=b1[:, :])
                g_ps = psS.tile([1, P], F32, tag="g", name=f"g{i}")
                nc.tensor.matmul(g_ps, lhsT=w2, rhs=hT, start=True, stop=True)
                # sigmoid(z+b2) = (1+tanh((z+b2)/2))/2; beta = 1+sigmoid
                t_i = work.tile([1, P], F32, tag="bi", name=f"bi{i}")
                nc.scalar.activation(
                    out=t_i, in_=g_ps, func=AF.Tanh, scale=0.5, bias=b2h[:, :]
                )
                bc_ps = psS.tile([P, 1], F32, tag="bc", name=f"bc{i}")
                nc.tensor.transpose(bc_ps, t_i, ident[:1, :1])
                # beta = 1.5 + 0.5*tanh : folded into the PSUM->SBUF copy
                nc.scalar.activation(
                    out=bcolbuf[:, t, i : i + 1], in_=bc_ps, func=AF.Copy,
                    scale=0.5, bias=1.5,
                )

            # KL phase
            for b, (ip, ig, w) in enumerate(((0, 2, WX), (1, 3, WY))):
                e = work.tile([P, w], F32, tag=f"e{b}", name=f"e{b}")
                nc.scalar.activation(
                    out=e, in_=xt[ig], func=AF.Exp,
                    scale=bcolbuf[:, t, ig : ig + 1],
                    accum_out=Zbuf[:, t, b : b + 1],
                )
                nc.scalar.activation(
                    out=cp[ip], in_=xt[ip], func=AF.Exp,
                    scale=bcolbuf[:, t, ip : ip + 1],
                    accum_out=Zbuf[:, t, 2 + b : 3 + b],
                )
                prodA = work.tile([P, w], F32, tag=f"prod{b}", name=f"prA{b}")
                nc.gpsimd.tensor_mul(prodA, e, xt[ig])
                nc.scalar.activation(
                    out=cp[ig], in_=prodA, func=AF.Copy, scale=1.0 / w,
                    accum_out=Abuf[:, t, b : b + 1],
                )
                prodB = work.tile([P, w], F32, tag=f"prod{b}", name=f"prB{b}")
                nc.gpsimd.tensor_mul(prodB, e, xt[ip])
                nc.scalar.activation(
                    out=cp[ip], in_=prodB, func=AF.Copy, scale=1.0 / w,
                    accum_out=Bbuf[:, t, b : b + 1],
                )

        # ---- epilogue: assemble loss rows for all tiles at once ----
        lnZ = singles.tile([P, NT, 4], F32)
        nc.scalar.activation(out=lnZ, in_=Zbuf, func=AF.Ln)
        rg = singles.tile([P, NT, 2], F32)
        nc.vector.reciprocal(out=rg, in_=Zbuf[:, :, 0:2])
        ta = singles.tile([P, NT, 2], F32)
        nc.vector.tensor_mul(ta, bcolbuf[:, :, 2:4], Abuf)     # bg*A
        tb = singles.tile([P, NT, 2], F32)
        nc.vector.tensor_mul(tb, bcolbuf[:, :, 0:2], Bbuf)     # bp*B
        nc.vector.tensor_sub(ta, ta, tb)
        nc.vector.tensor_mul(ta, ta, rg)                       # (bgA-bpB)/Zg
        u = singles.tile([P, NT, 2], F32)
        nc.vector.tensor_sub(u, lnZ[:, :, 2:4], lnZ[:, :, 0:2])  # lnZp-lnZg
        lsum = singles.tile([P, NT], F32)
        nc.vector.tensor_add(lsum, ta[:, :, 0], ta[:, :, 1])
        ux = singles.tile([P, NT], F32)
        nc.vector.tensor_scalar_mul(ux, u[:, :, 0], 1.0 / WX)
        nc.vector.tensor_add(lsum, lsum, ux)
        nc.vector.tensor_scalar_mul(ux, u[:, :, 1], 1.0 / WY)
        nc.vector.tensor_add(lsum, lsum, ux)
        nc.vector.tensor_mul(lsum, lsum, tw)
        accv = singles.tile([P, 1], F32)
        nc.vector.reduce_sum(out=accv, in_=lsum, axis=mybir.AxisListType.X)
        tot_ps = psS.tile([1, 1], F32, tag="tot")
        nc.tensor.matmul(tot_ps, lhsT=accv, rhs=ones, start=True, stop=True)
        res = singles.tile([1, 1], F32)
        nc.scalar.activation(out=res, in_=tot_ps, func=AF.Copy, scale=1.0 / K)
        nc.sync.dma_start(out=out_d[:, :], in_=res)

    split_excess_waits(nc)
    return nc


_NC_CACHE = {}


def _get_nc():
    if "nc" not in _NC_CACHE:
        _NC_CACHE["nc"] = build_nc()
    return _NC_CACHE["nc"]


def make_in_maps(inputs):
    in_maps = []
    for c in range(NCORES):
        sl = slice(c * BP, (c + 1) * BP)
        m = {
            "output_x": np.ascontiguousarray(
                inputs["output_x"][sl].reshape(ROWS, WX), np.float32),
            "output_y": np.ascontiguousarray(
                inputs["output_y"][sl].reshape(ROWS, WY), np.float32),
            "target_x": np.ascontiguousarray(
                inputs["target_x"][sl].reshape(ROWS, WX), np.float32),
            "target_y": np.ascontiguousarray(
                inputs["target_y"][sl].reshape(ROWS, WY), np.float32),
            "tw": np.ascontiguousarray(
                inputs["target_weight"][sl].reshape(NT, P).T, np.float32),
            "fcx_w1": np.ascontiguousarray(inputs["fcx_w1"], np.float32),
            "fcx_b1": np.ascontiguousarray(inputs["fcx_b1"].reshape(-1, 1), np.float32),
            "fcx_w2": np.ascontiguousarray(inputs["fcx_w2"], np.float32),
            "fcx_b2": np.ascontiguousarray(inputs["fcx_b2"].reshape(1, 1), np.float32),
            "fcy_w1": np.ascontiguousarray(inputs["fcy_w1"], np.float32),
            "fcy_b1": np.ascontiguousarray(inputs["fcy_b1"].reshape(-1, 1), np.float32),
            "fcy_w2": np.ascontiguousarray(inputs["fcy_w2"], np.float32),
            "fcy_b2": np.ascontiguousarray(inputs["fcy_b2"].reshape(1, 1), np.float32),
        }
        in_maps.append(m)
    return in_maps


def kernel(**inputs) -> np.ndarray:
    nc = _get_nc()
    in_maps = make_in_maps(inputs)
    res = run_bass_kernel_spmd(nc, in_maps, core_ids=list(range(NCORES)))
    total = np.float64(0.0)
    for c in range(NCORES):
        total += np.float64(res.results[c]["out"][0, 0])
    return np.float32(total)
